# revision 1
# baseline (speedup 1.0000x reference)
"""Trainium2 Bass kernel for NT-Xent / SimCLR contrastive loss, v2.

Design (8 cores, data-parallel over rows of z = concat(z_i, z_j)):
  Host rotates the concatenated embeddings by c*1024 rows per core, so
  every core runs the identical SPMD program on "local" rows: its own
  slab is local rows [0, 1024), the positives partner slab is local rows
  [4096, 5120).

  Per core, streaming 8 chunks of 1024 rows:
    1. SWDGE cast load fp32 -> bf16 (raw, unnormalized).
    2. DVE tensor_tensor_reduce (fused square+sum) -> row norms nsq;
       Newton rsqrt (no banned ACT Rsqrt).
    3. PE-array transposes raw bf16 128x128 blocks -> PSUM; Pool engine
       copies PSUM -> SBUF with a bf16 -> fp8e4 cast.  No DRAM bounce.
    4. fp8 DoubleRow matmuls (2x PE throughput, K=256/pass):
       stationary = raw transposed col-block [128, 2, 128], moving =
       8*normalized my-slab [128, 2, 1024] -> psum = 8*|e_c| * sim[c, m].
    5. ScalarE exp with PER-PARTITION scale AP (0.25 * rsqrt(nsq_c)):
       exp(2*sim) with the column normalization folded into the scale;
       fused accum_out gives column partials sum_m exp(2 sim[c, m]).
  Because sim is symmetric, summing the 8 cores' (un-rotated) column
  partials yields every row's full denominator; the host subtracts the
  diagonal exp(2) and adds the positives (computed on-device in fp32
  from normalized bf16 tiles via fused mul+reduce).
"""

import sys

if "/opt/trn_rl_repo" not in sys.path:
    sys.path.insert(0, "/opt/trn_rl_repo")

import numpy as np

N = 4096
D = 512
TEMP = 0.5
INV_T = 1.0 / TEMP

N2 = 2 * N            # 8192
NCORES = 8
ROWS = N2 // NCORES   # 1024 rows per core slab
P = 128
NCHUNK = 8            # row chunks per core
CH_T = ROWS // P      # 8 row-tiles (128 rows) per chunk
TB = N2 // P          # 64 row/col blocks total
KT = D // P           # 4 k slabs
SC = 8.0              # fp8 operand scale for the normalized slab

_CACHE = {}


def _newton_rsqrt(nc, mybir, s, tmp, nsq, iters=3):
    """s = 1/sqrt(nsq) on DVE only. Seed = tangent fit at nsq ~= D."""
    OP = mybir.AluOpType
    a = -0.5 * float(D) ** -1.5
    b = 1.5 * float(D) ** -0.5
    nc.vector.tensor_scalar(out=s, in0=nsq, scalar1=a, scalar2=b, op0=OP.mult, op1=OP.add)
    for _ in range(iters):
        nc.vector.tensor_mul(out=tmp, in0=s, in1=s)
        nc.vector.tensor_mul(out=tmp, in0=tmp, in1=nsq)
        nc.vector.tensor_scalar(
            out=tmp, in0=tmp, scalar1=-0.5, scalar2=1.5, op0=OP.mult, op1=OP.add
        )
        nc.vector.tensor_mul(out=s, in0=s, in1=tmp)


def build(debug=False):
    import concourse.bacc as bacc
    import concourse.tile as tile
    from concourse import mybir
    from concourse.masks import make_identity

    f32 = mybir.dt.float32
    bf16 = mybir.dt.bfloat16
    fp8 = mybir.dt.float8e4
    AF = mybir.ActivationFunctionType
    OP = mybir.AluOpType
    DR = mybir.MatmulPerfMode.DoubleRow

    nc = bacc.Bacc(
        "TRN2", target_bir_lowering=False, debug=debug, num_devices=NCORES
    )

    emb = nc.dram_tensor("emb", [N2, D], f32, kind="ExternalInput").ap()
    dsum_d = nc.dram_tensor("dsum", [P, TB], f32, kind="ExternalOutput").ap()
    pos_d = nc.dram_tensor("pos", [P, CH_T], f32, kind="ExternalOutput").ap()

    emb_t = emb.rearrange("(t p) d -> p t d", p=P)  # [128, 64, 512]

    with (
        tile.TileContext(nc) as tc,
        tc.tile_pool(name="persist", bufs=1) as persist,
        tc.tile_pool(name="stage", bufs=2) as stage,
        tc.tile_pool(name="small", bufs=2) as small,
        tc.tile_pool(name="mmps", bufs=2, space="PSUM") as mmps,
        tc.tile_pool(name="trps", bufs=3, space="PSUM") as trps,
    ):
        def mk(shape, dtype, name, pool=persist):
            return pool.tile(shape, dtype, name=name, tag=name)

        # persistent tiles
        zT8 = mk([P, KT, N2], fp8, "zT8")          # raw transposed, fp8
        m8T = mk([P, KT, ROWS], fp8, "m8T")        # 8*normalized my slab, transposed
        emy = mk([P, CH_T, D], bf16, "emy")        # 8*normalized my slab, row-major
        s_scale = mk([P, TB], f32, "s_scale")      # 0.25 * rsqrt(nsq) per local row
        dsum_sb = mk([P, TB], f32, "dsum_sb")
        pos_sb = mk([P, CH_T], f32, "pos_sb")
        ident = mk([P, P], bf16, "ident")
        dump0 = mk([P, D], bf16, "dump0")
        dump1 = mk([P, D], bf16, "dump1")
        make_identity(nc, ident)

        def prep(c):
            ech = stage.tile([P, CH_T, D], bf16, tag="ech", name=f"ech{c}")
            nc.gpsimd.dma_start(
                out=ech, in_=emb_t[:, c * CH_T : (c + 1) * CH_T, :]
            )
            # bn_stats norms (baseline path)
            nsq = small.tile([P, CH_T], f32, tag="nsq", name=f"nsq{c}")
            mv = small.tile([P, 2 * CH_T], f32, tag="mv", name=f"mv{c}")
            bnst = small.tile([P, 6], f32, tag="bnst", name=f"bnst{c}")
            for t in range(CH_T):
                nc.vector.bn_stats(out=bnst, in_=ech[:, t, :])
                nc.vector.bn_aggr(out=mv[:, 2 * t : 2 * t + 2], in_=bnst)
            mv3 = mv.rearrange("p (t two) -> p t two", two=2)
            nsq3 = nsq.rearrange("p (t one) -> p t one", one=1)
            nc.vector.tensor_mul(out=nsq3, in0=mv3[:, :, 0:1], in1=mv3[:, :, 0:1])
            nc.vector.tensor_add(out=nsq3, in0=nsq3, in1=mv3[:, :, 1:2])
            nc.vector.tensor_scalar_mul(out=nsq, in0=nsq, scalar1=float(D))
            r = small.tile([P, CH_T], f32, tag="r", name=f"r{c}")
            tmp = small.tile([P, CH_T], f32, tag="tmp", name=f"tmp{c}")
            _newton_rsqrt(nc, mybir, r, tmp, nsq)
            nc.vector.tensor_scalar_mul(
                out=s_scale[:, c * CH_T : (c + 1) * CH_T], in0=r, scalar1=INV_T / SC
            )

            if c == 0:
                # normalized (x8) my slab: row-major for positives, and
                # transposed fp8 as the moving matmul operand.
                s8 = small.tile([P, CH_T], f32, tag="s8", name="s8_0")
                nc.vector.tensor_scalar_mul(out=s8, in0=r, scalar1=SC)
                for t in range(CH_T):
                    nc.vector.tensor_scalar_mul(
                        out=emy[:, t, :], in0=ech[:, t, :], scalar1=s8[:, t : t + 1]
                    )
                for t in range(CH_T):
                    trt = trps.tile([P, KT, P], bf16, tag="trt", name=f"trtm{t}")
                    for k in range(KT):
                        nc.tensor.transpose(
                            trt[:, k, :], emy[:, t, k * P : (k + 1) * P], ident
                        )
                    nc.vector.tensor_copy(
                        out=m8T[:, :, t * P : (t + 1) * P], in_=trt
                    )
            if c == 4:
                # positives: pos64 = (8 z_my).(8 z_pair) = 64 * pos
                s8 = small.tile([P, CH_T], f32, tag="s8", name="s8_4")
                nc.vector.tensor_scalar_mul(out=s8, in0=r, scalar1=SC)
                epr = stage.tile([P, CH_T, D], bf16, tag="epr", name="epr")
                for t in range(CH_T):
                    nc.vector.tensor_scalar_mul(
                        out=epr[:, t, :], in0=ech[:, t, :], scalar1=s8[:, t : t + 1]
                    )
                prodf = stage.tile([P, D], f32, tag="prodf", name="prodf")
                for t in range(CH_T):
                    nc.vector.tensor_mul(out=prodf, in0=emy[:, t, :], in1=epr[:, t, :])
                    nc.vector.tensor_reduce(
                        out=pos_sb[:, t : t + 1], in_=prodf,
                        axis=mybir.AxisListType.X, op=OP.add,
                    )
            return ech

        def prep_tile(c, ech, t):
            # 4 transposes of one row-tile -> psum -> fp8 copy to zT8
            trt = trps.tile([P, KT, P], bf16, tag="trt", name=f"trt{c}_{t}")
            for k in range(KT):
                nc.tensor.transpose(
                    trt[:, k, :], ech[:, t, k * P : (k + 1) * P], ident
                )
            g = (c * CH_T + t) * P
            if t % 3 == 2:
                nc.scalar.activation(
                    out=zT8[:, :, g : g + P], in_=trt, func=AF.Copy
                )
            else:
                nc.vector.tensor_copy(out=zT8[:, :, g : g + P], in_=trt)

        def mm_exp_one(j):
            ps = mmps.tile([P, ROWS], f32, tag="ps", name=f"ps{j}")
            for kp in range(2):
                for h in range(2):
                    nc.tensor.matmul(
                        ps[:, h * 512 : (h + 1) * 512],
                        zT8[:, 2 * kp : 2 * kp + 2, j * P : (j + 1) * P],
                        m8T[:, 2 * kp : 2 * kp + 2, h * 512 : (h + 1) * 512],
                        start=(kp == 0),
                        stop=(kp == 1),
                        perf_mode=DR,
                    )
            nc.scalar.activation(
                out=ps,
                in_=ps,
                func=AF.Exp,
                scale=s_scale[:, j : j + 1],
                accum_out=dsum_sb[:, j : j + 1],
            )

        ech0 = prep(0)
        for t in range(CH_T):
            prep_tile(0, ech0, t)
        ech1 = prep(1)
        for t in range(CH_T):
            prep_tile(1, ech1, t)
        for c in range(NCHUNK):
            echn = prep(c + 2) if c + 2 < NCHUNK else None
            # interleave next chunk's transposes between this chunk's matmuls
            for t in range(CH_T):
                if echn is not None:
                    prep_tile(c + 2, echn, t)
                mm_exp_one(c * CH_T + t)

        nc.sync.dma_start(out=dsum_d, in_=dsum_sb)
        nc.sync.dma_start(out=pos_d, in_=pos_sb)

    nc.compile()
    return nc


def _get_nc():
    if "nc" not in _CACHE:
        _CACHE["nc"] = build()
    return _CACHE["nc"]


def make_in_maps(emb_i, emb_j):
    z_cat = np.ascontiguousarray(
        np.concatenate([emb_i, emb_j], axis=0), dtype=np.float32
    )
    in_maps = []
    for c in range(NCORES):
        r0 = c * ROWS
        rot = np.ascontiguousarray(np.concatenate([z_cat[r0:], z_cat[:r0]], axis=0))
        in_maps.append({"emb": rot})
    return in_maps


def finish_host(results):
    """Combine per-core column partials + positives into the scalar loss."""
    denom = np.zeros(N2, dtype=np.float64)
    pos = np.zeros(N2, dtype=np.float64)
    for c in range(NCORES):
        dsumT = results[c]["dsum"].astype(np.float64)   # [128, 64]
        colpart_local = dsumT.T.reshape(N2)             # local row j*128+p
        denom += np.roll(colpart_local, c * ROWS)       # un-rotate
        p64 = results[c]["pos"].astype(np.float64)      # [128, 8]
        pos[c * ROWS : (c + 1) * ROWS] = p64.T.reshape(ROWS) / (SC * SC)
    denom -= np.exp(INV_T)                              # drop diagonal term
    loss = np.log(denom) - INV_T * pos
    return np.float32(loss.sum() / N2)


def kernel(emb_i, emb_j):
    from concourse.bass_utils import run_bass_kernel_spmd

    nc = _get_nc()
    in_maps = make_in_maps(np.asarray(emb_i), np.asarray(emb_j))
    try:
        res = run_bass_kernel_spmd(nc, in_maps, core_ids=list(range(NCORES)))
    except Exception:
        res = run_bass_kernel_spmd(nc, in_maps, core_ids=list(range(NCORES)))
    _CACHE["last_results"] = res
    return finish_host(res.results)



# revision 2
# speedup vs baseline: 1.8647x; 1.8647x over previous
"""Trainium2 Bass kernel for NT-Xent / SimCLR contrastive loss, v3.

Design (8 cores, data-parallel over rows of z = concat(z_i, z_j)):
  Host pre-normalizes z (L2 rows), scales by 8, transposes to feature-
  major [512, 8192], casts to fp8e4, and rotates by c*1024 columns per
  core so every core's own rows sit at columns [0, 1024).

  Device per core (pure matmul + exp pipeline, no transposes/casts):
    - DMA the fp8 [128, 4, 8192] operand array in 8 column chunks.
    - For each of my 8 row-blocks t (stationary = zt8[:, :, t*128:+128]):
      sweep all 8192 columns in 4 chunks of 2048 (moving operand),
      K=512 via 2 DoubleRow passes -> psum[128, 2048] = 64*sim.
    - ScalarE exp(0.03125 * psum) in place with accum_out -> complete
      row-sum of exp(2*sim[r, :]) per partition; 32 partials [128, 4t+s].
  Host: denom[r] = sum_s dsum[...] - exp(2); positives from fp32 z;
  loss = mean(log(denom) - 2*pos).
"""

import sys

if "/opt/trn_rl_repo" not in sys.path:
    sys.path.insert(0, "/opt/trn_rl_repo")

import numpy as np

N = 4096
D = 512
TEMP = 0.5
INV_T = 1.0 / TEMP

N2 = 2 * N            # 8192
NCORES = 8
ROWS = N2 // NCORES   # 1024 rows per core
P = 128
MT = ROWS // P        # 8 stationary row-blocks per core
SW = 2048             # moving sweep chunk (4 psum banks)
NSW = N2 // SW        # 4 sweep chunks
KP = 2                # DoubleRow K passes (256 features each)
SC = 8.0              # fp8 operand scale; psum = SC*SC*sim

_CACHE = {}


def build(debug=False):
    import concourse.bacc as bacc
    import concourse.tile as tile
    from concourse import mybir

    f32 = mybir.dt.float32
    fp8 = mybir.dt.float8e4
    AF = mybir.ActivationFunctionType
    DR = mybir.MatmulPerfMode.DoubleRow

    nc = bacc.Bacc(
        "TRN2", target_bir_lowering=False, debug=debug, num_devices=NCORES
    )

    zt_d = nc.dram_tensor("zt", [D, N2], fp8, kind="ExternalInput").ap()
    dsum_d = nc.dram_tensor("dsum", [P, MT * NSW], f32, kind="ExternalOutput").ap()

    zt_t = zt_d.rearrange("(k p) r -> p k r", p=P)  # [128, 4, 8192]

    with (
        tile.TileContext(nc) as tc,
        tc.tile_pool(name="persist", bufs=1) as persist,
        tc.tile_pool(name="mmps", bufs=2, space="PSUM") as mmps,
    ):
        zt8 = persist.tile([P, D // P, N2], fp8, name="zt8", tag="zt8")
        acc = persist.tile([P, MT * NSW], f32, name="acc", tag="acc")

        # load the fp8 operand array in 8 column chunks on 2 queues
        CH = N2 // 8
        for c in range(8):
            eng = nc.gpsimd if c % 2 == 0 else nc.sync
            eng.dma_start(
                out=zt8[:, :, c * CH : (c + 1) * CH],
                in_=zt_t[:, :, c * CH : (c + 1) * CH],
            )

        for t in range(MT):
            for s in range(NSW):
                ps = mmps.tile([P, SW], f32, tag="ps", name=f"ps{t}_{s}")
                for kp in range(KP):
                    for h in range(SW // 512):
                        m0 = s * SW + h * 512
                        nc.tensor.matmul(
                            ps[:, h * 512 : (h + 1) * 512],
                            zt8[:, 2 * kp : 2 * kp + 2, t * P : (t + 1) * P],
                            zt8[:, 2 * kp : 2 * kp + 2, m0 : m0 + 512],
                            start=(kp == 0),
                            stop=(kp == KP - 1),
                            perf_mode=DR,
                        )
                nc.scalar.activation(
                    out=ps,
                    in_=ps,
                    func=AF.Exp,
                    scale=float(INV_T / (SC * SC)),
                    accum_out=acc[:, t * NSW + s : t * NSW + s + 1],
                )

        nc.sync.dma_start(out=dsum_d, in_=acc)

    nc.compile()
    return nc


def _get_nc():
    if "nc" not in _CACHE:
        _CACHE["nc"] = build()
    return _CACHE["nc"]


def _prep_host(emb_i, emb_j):
    """Normalize, scale, transpose, cast fp8; return (zt8_full, z, pos)."""
    import ml_dtypes

    z = np.concatenate(
        [np.asarray(emb_i, dtype=np.float32), np.asarray(emb_j, dtype=np.float32)],
        axis=0,
    )
    nrm = np.maximum(np.sqrt((z * z).sum(axis=1)), 1e-12)
    z /= nrm[:, None]
    pos = (z[:N] * z[N:]).sum(axis=1, dtype=np.float64)   # [N]
    zt8 = (SC * z.T).astype(ml_dtypes.float8_e4m3)        # [512, 8192]
    return zt8, pos


def make_in_maps(emb_i, emb_j):
    zt8, pos = _prep_host(emb_i, emb_j)
    _CACHE["pos"] = pos
    in_maps = []
    for c in range(NCORES):
        rot = np.ascontiguousarray(np.roll(zt8, -c * ROWS, axis=1))
        in_maps.append({"zt": rot})
    return in_maps


def finish_host(results):
    """Assemble per-core row denominators into the scalar loss."""
    denom = np.empty(N2, dtype=np.float64)
    for c in range(NCORES):
        d = results[c]["dsum"].astype(np.float64)          # [128, 32]
        # row (t*128 + p) local = global c*1024 + t*128 + p
        rows = d.reshape(P, MT, NSW).sum(axis=2)           # [128, 8]
        denom[c * ROWS : (c + 1) * ROWS] = rows.T.reshape(ROWS)
    denom -= np.exp(INV_T)                                 # drop diagonal term
    pos = _CACHE["pos"]
    loss = np.log(denom) - INV_T * np.concatenate([pos, pos])
    return np.float32(loss.sum() / N2)


def kernel(emb_i, emb_j):
    from concourse.bass_utils import run_bass_kernel_spmd

    nc = _get_nc()
    in_maps = make_in_maps(np.asarray(emb_i), np.asarray(emb_j))
    try:
        res = run_bass_kernel_spmd(nc, in_maps, core_ids=list(range(NCORES)))
    except Exception:
        res = run_bass_kernel_spmd(nc, in_maps, core_ids=list(range(NCORES)))
    _CACHE["last_results"] = res
    return finish_host(res.results)


# revision 3
# speedup vs baseline: 1.8887x; 1.0129x over previous
"""Trainium2 Bass kernel for NT-Xent / SimCLR contrastive loss, v3.

Design (8 cores, data-parallel over rows of z = concat(z_i, z_j)):
  Host pre-normalizes z (L2 rows), scales by 8, transposes to feature-
  major [512, 8192], casts to fp8e4, and rotates by c*1024 columns per
  core so every core's own rows sit at columns [0, 1024).

  Device per core (pure matmul + exp pipeline, no transposes/casts):
    - DMA the fp8 [128, 4, 8192] operand array in 8 column chunks.
    - For each of my 8 row-blocks t (stationary = zt8[:, :, t*128:+128]):
      sweep all 8192 columns in 4 chunks of 2048 (moving operand),
      K=512 via 2 DoubleRow passes -> psum[128, 2048] = 64*sim.
    - ScalarE exp(0.03125 * psum) in place with accum_out -> complete
      row-sum of exp(2*sim[r, :]) per partition; 32 partials [128, 4t+s].
  Host: denom[r] = sum_s dsum[...] - exp(2); positives from fp32 z;
  loss = mean(log(denom) - 2*pos).
"""

import sys

if "/opt/trn_rl_repo" not in sys.path:
    sys.path.insert(0, "/opt/trn_rl_repo")

import numpy as np

N = 4096
D = 512
TEMP = 0.5
INV_T = 1.0 / TEMP

N2 = 2 * N            # 8192
NCORES = 8
ROWS = N2 // NCORES   # 1024 rows per core
P = 128
MT = ROWS // P        # 8 stationary row-blocks per core
SW = 2048             # moving sweep chunk (4 psum banks)
NSW = N2 // SW        # 4 sweep chunks
KP = 2                # DoubleRow K passes (256 features each)
SC = 8.0              # fp8 operand scale; psum = SC*SC*sim

_CACHE = {}


def build(debug=False):
    import concourse.bacc as bacc
    import concourse.tile as tile
    from concourse import mybir

    f32 = mybir.dt.float32
    fp8 = mybir.dt.float8e4
    AF = mybir.ActivationFunctionType
    DR = mybir.MatmulPerfMode.DoubleRow

    nc = bacc.Bacc(
        "TRN2", target_bir_lowering=False, debug=debug, num_devices=NCORES
    )

    zt_d = nc.dram_tensor("zt", [D, N2], fp8, kind="ExternalInput").ap()
    dsum_d = nc.dram_tensor("dsum", [P, MT * NSW], f32, kind="ExternalOutput").ap()

    zt_t = zt_d.rearrange("(k p) r -> p k r", p=P)  # [128, 4, 8192]

    with (
        tile.TileContext(nc) as tc,
        tc.tile_pool(name="persist", bufs=1) as persist,
        tc.tile_pool(name="mmps", bufs=2, space="PSUM") as mmps,
    ):
        zt8 = persist.tile([P, D // P, N2], fp8, name="zt8", tag="zt8")
        acc = persist.tile([P, MT * NSW], f32, name="acc", tag="acc")

        # load the fp8 operand array in 16 column chunks, alternating the
        # two hardware-DGE rings (sync + scalar); SWDGE (gpsimd) trickles.
        NCH = 16
        CH = N2 // NCH
        for c in range(NCH):
            eng = nc.sync if c % 2 == 0 else nc.scalar
            eng.dma_start(
                out=zt8[:, :, c * CH : (c + 1) * CH],
                in_=zt_t[:, :, c * CH : (c + 1) * CH],
            )

        for t in range(MT):
            for s in range(NSW):
                ps = mmps.tile([P, SW], f32, tag="ps", name=f"ps{t}_{s}")
                for kp in range(KP):
                    for h in range(SW // 512):
                        m0 = s * SW + h * 512
                        nc.tensor.matmul(
                            ps[:, h * 512 : (h + 1) * 512],
                            zt8[:, 2 * kp : 2 * kp + 2, t * P : (t + 1) * P],
                            zt8[:, 2 * kp : 2 * kp + 2, m0 : m0 + 512],
                            start=(kp == 0),
                            stop=(kp == KP - 1),
                            perf_mode=DR,
                        )
                nc.scalar.activation(
                    out=ps,
                    in_=ps,
                    func=AF.Exp,
                    scale=float(INV_T / (SC * SC)),
                    accum_out=acc[:, t * NSW + s : t * NSW + s + 1],
                )

        nc.sync.dma_start(out=dsum_d, in_=acc)

    nc.compile()
    return nc


def _get_nc():
    if "nc" not in _CACHE:
        _CACHE["nc"] = build()
    return _CACHE["nc"]


def _prep_host(emb_i, emb_j):
    """Normalize, scale, transpose, cast fp8; return (zt8_full, z, pos)."""
    import ml_dtypes

    z = np.concatenate(
        [np.asarray(emb_i, dtype=np.float32), np.asarray(emb_j, dtype=np.float32)],
        axis=0,
    )
    nrm = np.maximum(np.sqrt((z * z).sum(axis=1)), 1e-12)
    z /= nrm[:, None]
    pos = (z[:N] * z[N:]).sum(axis=1, dtype=np.float64)   # [N]
    zt8 = (SC * z.T).astype(ml_dtypes.float8_e4m3)        # [512, 8192]
    return zt8, pos


def make_in_maps(emb_i, emb_j):
    zt8, pos = _prep_host(emb_i, emb_j)
    _CACHE["pos"] = pos
    in_maps = []
    for c in range(NCORES):
        rot = np.ascontiguousarray(np.roll(zt8, -c * ROWS, axis=1))
        in_maps.append({"zt": rot})
    return in_maps


def finish_host(results):
    """Assemble per-core row denominators into the scalar loss."""
    denom = np.empty(N2, dtype=np.float64)
    for c in range(NCORES):
        d = results[c]["dsum"].astype(np.float64)          # [128, 32]
        # row (t*128 + p) local = global c*1024 + t*128 + p
        rows = d.reshape(P, MT, NSW).sum(axis=2)           # [128, 8]
        denom[c * ROWS : (c + 1) * ROWS] = rows.T.reshape(ROWS)
    denom -= np.exp(INV_T)                                 # drop diagonal term
    pos = _CACHE["pos"]
    loss = np.log(denom) - INV_T * np.concatenate([pos, pos])
    return np.float32(loss.sum() / N2)


def kernel(emb_i, emb_j):
    from concourse.bass_utils import run_bass_kernel_spmd

    nc = _get_nc()
    in_maps = make_in_maps(np.asarray(emb_i), np.asarray(emb_j))
    try:
        res = run_bass_kernel_spmd(nc, in_maps, core_ids=list(range(NCORES)))
    except Exception:
        res = run_bass_kernel_spmd(nc, in_maps, core_ids=list(range(NCORES)))
    _CACHE["last_results"] = res
    return finish_host(res.results)
